# revision 7
# baseline (speedup 1.0000x reference)
"""Trainium2 Bass kernel for nn_MeshUpConv (3-layer spline-conv GNN).

Fully fused: ONE NEFF execution runs all 3 layers on 8 NeuronCores (SPMD).

Sharding: nodes by range (6250/core); edges by destination core. Host packs
each core's edges into T tiles of 128 edge slots; tile t owns 16 node ids
[16t, 16t+16) (win16, no straddling).

Per layer on device:
  xjt   = ap_gather (gpsimd) from a channel-major SBUF table of uint32
          elements packing (fp16 value, dead lane); band-split across
          partition groups keeps every index < 32767 (int16).
  P     = xjt_hi.T @ W      (PE, fp16 via stride-2 views -> fp32 PSUM)
  Tp    = fp16(P * B[e,k])  (DVE STT, or Act copy + DVE multiply — balanced)
  agg  += Tp_k.T @ A_tile   (PE, 9 accumulating matmuls, 16-wide windows)
  agg  += root.T @ own_prev (PE, per group, from a per-core root table)
  relu  -> interleaved fp16 staging -> DRAM bounce (u32)

Between layers: AllGather of the (fp16,?) u32 node sections through DRAM,
then DMA the 8 sections into the gather table's band rows. Layer-3 output
is flushed straight to the output tensor in fp32.
"""
import sys
import numpy as np

sys.path.insert(0, "/opt/trn_rl_repo")

N_NODES = 50000
N_EDGES = 400000
C_IN = 64
C_OUT = 32
K = 9
N_CORES = 8
NODES_PER_CORE = N_NODES // N_CORES
TILE_E = 128
IDS_PER_TILE = 16
WIN = 16
TILES_PER_GROUP = 32
GROUP = IDS_PER_TILE * TILES_PER_GROUP  # 512
CHUNK = 16  # tiles per ap_gather


# ----------------------------------------------------------------------------
# Host preprocessing
# ----------------------------------------------------------------------------

def spline_basis_np(pseudo):
    def quad(t):
        return np.stack([0.5 * (1.0 - t) ** 2, -t * t + t + 0.5, 0.5 * t * t],
                        axis=-1)
    q0 = quad(pseudo[:, 0])
    q1 = quad(pseudo[:, 1])
    return (q1[:, :, None] * q0[:, None, :]).reshape(-1, K)


def pack16(a):
    """fp32 -> uint32 whose low 16 bits are the fp16 value (even lane)."""
    h = np.asarray(a, np.float32).astype(np.float16)
    return h.view(np.uint16).astype(np.uint32)


def _wrap(idx_flat):
    """[S] -> [16, S/16]: position i lives at (row i%16, col i//16)."""
    s = len(idx_flat)
    return np.ascontiguousarray(
        idx_flat.reshape(s // 16, 16).T).astype(np.int16)


def _pack_core(dst_local, n_nodes):
    """win16 first-fit, no straddling. Returns per-node (tile, offset)."""
    deg = np.bincount(dst_local, minlength=n_nodes)
    assert deg.max() <= TILE_E, "node degree exceeds one tile"
    tile_of = np.empty(n_nodes, np.int64)
    off_of = np.empty(n_nodes, np.int64)
    t = ids = slots = 0
    for v in range(n_nodes):
        d = deg[v]
        if ids == IDS_PER_TILE or slots + d > TILE_E:
            t += 1
            ids = slots = 0
        tile_of[v] = t
        off_of[v] = ids
        ids += 1
        slots += d
    return tile_of, off_of, t + 1


def build_static(edge_index, pseudo):
    src = np.asarray(edge_index[0], dtype=np.int64)
    dst = np.asarray(edge_index[1], dtype=np.int64)
    B = spline_basis_np(np.asarray(pseudo, dtype=np.float32))

    pre = []
    for c in range(N_CORES):
        lo_n = c * NODES_PER_CORE
        sel = np.nonzero((dst >= lo_n) & (dst < lo_n + NODES_PER_CORE))[0]
        dl = dst[sel] - lo_n
        order = np.argsort(dl, kind="stable")
        sel = sel[order]
        dl = dl[order]
        tile_of, off_of, ntiles = _pack_core(dl, NODES_PER_CORE)
        pre.append((sel, dl, tile_of, off_of, ntiles))

    t_max = max(p[4] for p in pre)
    T = ((t_max + TILES_PER_GROUP - 1) // TILES_PER_GROUP) * TILES_PER_GROUP
    idt = T * IDS_PER_TILE
    S = T * TILE_E
    assert 4 * idt + 1 < 32768, "table index overflows int16"

    # per-node global permuted column (0-based within the owner core)
    perms = []
    for c in range(N_CORES):
        _, _, tile_of, off_of, _ = pre[c]
        perms.append(tile_of * IDS_PER_TILE + off_of)

    cores = []
    for c in range(N_CORES):
        sel, dl, tile_of, off_of, _ = pre[c]
        e_src = src[sel]
        e_b = B[sel]
        e_tile = tile_of[dl]
        # position within tile (edges are sorted by dst, tiles in order)
        first_of_tile = np.zeros(T + 1, np.int64)
        np.add.at(first_of_tile, e_tile + 1, 1)
        first_idx = np.cumsum(first_of_tile)[:-1]  # first edge index per tile
        pos = np.arange(len(sel)) - first_idx[e_tile]
        slot = e_tile * TILE_E + pos

        a_pm = np.zeros((TILE_E, T * IDS_PER_TILE), np.float16)
        a_pm[pos, e_tile * IDS_PER_TILE + off_of[dl]] = 1.0
        b_pm = np.zeros((TILE_E, T * IDS_PER_TILE), np.float16)
        b_pm[pos[:, None],
             (e_tile * IDS_PER_TILE)[:, None] + np.arange(K)[None, :]] = e_b

        owner = e_src // NODES_PER_CORE
        local = e_src % NODES_PER_CORE
        qcol = np.empty(len(sel), np.int64)
        for o in range(N_CORES):
            m = owner == o
            qcol[m] = perms[o][local[m]]

        # layers 1-2: 2 bands of 64 partitions (cores 0-3 | 4-7)
        colA = np.zeros(S, np.int64)
        colB = np.zeros(S, np.int64)
        mA = owner < 4
        colA[slot[mA]] = (owner[mA] % 4) * idt + qcol[mA] + 1
        colB[slot[~mA]] = (owner[~mA] % 4) * idt + qcol[~mA] + 1
        idx12 = np.concatenate([np.tile(_wrap(colA), (4, 1)),
                                np.tile(_wrap(colB), (4, 1))])
        # layer 3: 4 bands of 32 partitions (core pairs)
        rows3 = []
        for q in range(4):
            colq = np.zeros(S, np.int64)
            m = owner // 2 == q
            colq[slot[m]] = (owner[m] % 2) * idt + qcol[m] + 1
            rows3.append(np.tile(_wrap(colq), (2, 1)))
        idx3 = np.concatenate(rows3)

        cores.append(dict(a_pm=a_pm, b_pm=b_pm, idx12=idx12, idx3=idx3,
                          perm=perms[c], node_lo=c * NODES_PER_CORE))
    return cores, perms, T, idt


def build_tables(x, skip, perms, idt):
    """Global gather tables (identical on every core) + per-core root tabs."""
    nel12 = 4 * idt + 1
    table_x = np.zeros((128, nel12), np.uint32)
    skip_tab = np.zeros((64, nel12), np.uint32)
    for o in range(N_CORES):
        sec = x[o * NODES_PER_CORE:(o + 1) * NODES_PER_CORE]
        ssec = skip[o * NODES_PER_CORE:(o + 1) * NODES_PER_CORE]
        band = o // 4
        cols = (o % 4) * idt + perms[o] + 1
        table_x[band * 64:band * 64 + C_IN, cols] = pack16(sec.T)
        skip_tab[band * 32:band * 32 + C_OUT, cols] = pack16(ssec.T)
    x_own, skip_own = [], []
    for c in range(N_CORES):
        xo = np.zeros((C_IN, idt), np.uint32)
        xo[:, perms[c]] = pack16(
            x[c * NODES_PER_CORE:(c + 1) * NODES_PER_CORE].T)
        so = np.zeros((C_OUT, idt), np.uint32)
        so[:, perms[c]] = pack16(
            skip[c * NODES_PER_CORE:(c + 1) * NODES_PER_CORE].T)
        x_own.append(xo)
        skip_own.append(so)
    return table_x, skip_tab, x_own, skip_own


def build_weights(W1, root1, b1, W2, root2, b2):
    w1c = np.ascontiguousarray(
        W1.transpose(1, 0, 2).reshape(C_IN, K * C_OUT)).astype(np.float16)
    w2c = np.ascontiguousarray(
        W2.transpose(1, 0, 2).reshape(C_OUT, K * C_OUT)).astype(np.float16)
    wsp1 = np.tile(w1c, (2, 1))                  # [128, 288] layer 1 (2 bands)
    wsp2 = np.tile(w1c, (2, 1))                  # [128, 288] layer 2 (2 bands)
    wsp3 = np.tile(w2c, (4, 1))                  # [128, 288] layer 3 (4 bands)
    rta1 = root1.astype(np.float16)              # [64, 32]
    rta2 = root1.astype(np.float16)              # [64, 32] (h,skip rows match)
    rta3 = root2.astype(np.float16)              # [32, 32]
    return (wsp1, wsp2, wsp3, rta1, rta2, rta3,
            b1.reshape(C_OUT, 1).astype(np.float32),
            b2.reshape(C_OUT, 1).astype(np.float32))


def unpermute(outT_list, cores):
    res = np.zeros((N_NODES, C_OUT), dtype=np.float32)
    for cc, o in zip(cores, outT_list):
        res[cc["node_lo"]:cc["node_lo"] + NODES_PER_CORE] = \
            o[:, cc["perm"]].T
    return res


# ----------------------------------------------------------------------------
# Bass kernel
# ----------------------------------------------------------------------------

def build_bass_kernel(T, idt):
    import concourse.bass as bass
    import concourse.bacc as bacc
    import concourse.mybir as mybir
    import concourse.tile as tile

    fp16 = mybir.dt.float16
    fp32 = mybir.dt.float32
    u32 = mybir.dt.uint32
    i16 = mybir.dt.int16
    nel12 = 4 * idt + 1
    nel3 = 2 * idt + 1
    S = T * TILE_E
    n_groups = T // TILES_PER_GROUP

    nc = bacc.Bacc("TRN2", target_bir_lowering=False, debug=False,
                   enable_asserts=False, num_devices=N_CORES)

    table_d = nc.dram_tensor("table_x", [128, nel12], u32, kind="ExternalInput")
    skip_d = nc.dram_tensor("skip_tab", [64, nel12], u32, kind="ExternalInput")
    xown_d = nc.dram_tensor("x_own", [C_IN, idt], u32, kind="ExternalInput")
    sown_d = nc.dram_tensor("skip_own", [C_OUT, idt], u32,
                            kind="ExternalInput")
    idx12_d = nc.dram_tensor("idx12", [128, S // 16], i16, kind="ExternalInput")
    idx3_d = nc.dram_tensor("idx3", [128, S // 16], i16, kind="ExternalInput")
    a_d = nc.dram_tensor("a_pm", [TILE_E, T * IDS_PER_TILE], fp16,
                         kind="ExternalInput")
    b_d = nc.dram_tensor("b_pm", [TILE_E, T * IDS_PER_TILE], fp16,
                         kind="ExternalInput")
    wsp1_d = nc.dram_tensor("wsp1", [128, K * C_OUT], fp16,
                            kind="ExternalInput")
    wsp2_d = nc.dram_tensor("wsp2", [128, K * C_OUT], fp16,
                            kind="ExternalInput")
    wsp3_d = nc.dram_tensor("wsp3", [128, K * C_OUT], fp16,
                            kind="ExternalInput")
    rta1_d = nc.dram_tensor("rta1", [C_IN, C_OUT], fp16, kind="ExternalInput")
    rta2_d = nc.dram_tensor("rta2", [C_IN, C_OUT], fp16, kind="ExternalInput")
    rta3_d = nc.dram_tensor("rta3", [C_OUT, C_OUT], fp16, kind="ExternalInput")
    b1_d = nc.dram_tensor("bias1", [C_OUT, 1], fp32, kind="ExternalInput")
    b2_d = nc.dram_tensor("bias2", [C_OUT, 1], fp32, kind="ExternalInput")
    outt = nc.dram_tensor("outt", [C_OUT, idt], fp32, kind="ExternalOutput")

    with tile.TileContext(nc) as tc:
        with (
            tc.tile_pool(name="const", bufs=1) as cpool,
            tc.tile_pool(name="gath", bufs=2) as gpool,
            tc.tile_pool(name="tp", bufs=4) as tppool,
            tc.tile_pool(name="st", bufs=2) as spool,
            tc.tile_pool(name="psP", bufs=2, space="PSUM") as psP,
            tc.tile_pool(name="psA", bufs=2, space="PSUM") as psA,
            tc.tile_pool(name="dram", bufs=1, space="DRAM") as dpool,
        ):
            table = cpool.tile([128, nel12], u32)
            nc.sync.dma_start(out=table[:], in_=table_d[:, :])
            root_tab = cpool.tile([C_IN, idt], u32)
            nc.sync.dma_start(out=root_tab[:], in_=xown_d[:, :])
            idx12 = cpool.tile([128, S // 16], i16)
            nc.sync.dma_start(out=idx12[:], in_=idx12_d[:, :])
            idx3 = cpool.tile([128, S // 16], i16)
            nc.sync.dma_start(out=idx3[:], in_=idx3_d[:, :])
            a_pm = cpool.tile([TILE_E, T * IDS_PER_TILE], fp16)
            nc.sync.dma_start(out=a_pm[:], in_=a_d[:, :])
            b_pm = cpool.tile([TILE_E, T * IDS_PER_TILE], fp16)
            nc.sync.dma_start(out=b_pm[:], in_=b_d[:, :])
            wsps, rtas, biases = [], [], []
            for wd, rd, bd, wrows, rrows in (
                    (wsp1_d, rta1_d, b1_d, 128, C_IN),
                    (wsp2_d, rta2_d, b1_d, 128, C_IN),
                    (wsp3_d, rta3_d, b2_d, 128, C_OUT)):
                w = cpool.tile([wrows, K * C_OUT], fp16)
                nc.sync.dma_start(out=w[:], in_=wd[:, :])
                r = cpool.tile([rrows, C_OUT], fp16)
                nc.sync.dma_start(out=r[:], in_=rd[:, :])
                bb = cpool.tile([C_OUT, 1], fp32)
                nc.sync.dma_start(out=bb[:], in_=bd[:, :])
                wsps.append(w)
                rtas.append(r)
                biases.append(bb)

            bounce_in = dpool.tile([C_OUT, idt], u32)
            bounce_out = dpool.tile([N_CORES, C_OUT, idt], u32)

            def hi(ap_u32):  # stride-2 fp16 view selecting the low lanes
                f = ap_u32.bitcast(fp16)
                n = f.shape[-1]
                return f.rearrange("p (n two) -> p n two", two=2)

            for layer in range(3):
                nel = nel12 if layer < 2 else nel3
                idx = idx12 if layer < 2 else idx3
                rrows = C_IN if layer < 2 else C_OUT
                wsp = wsps[layer]
                rta = rtas[layer]
                bias = biases[layer]
                rt16 = hi(root_tab[0:rrows, :])  # [rrows, idt, 2]
                for g in range(n_groups):
                    agg = psA.tile([C_OUT, GROUP], fp32)
                    nc.tensor.matmul(
                        agg[:], lhsT=rta[:],
                        rhs=rt16[:, g * GROUP:(g + 1) * GROUP, 0],
                        start=True, stop=False, skip_group_check=True)
                    for tt in range(TILES_PER_GROUP):
                        t = g * TILES_PER_GROUP + tt
                        if t % CHUNK == 0:
                            gb = gpool.tile([128, CHUNK * TILE_E], u32)
                            nc.gpsimd.ap_gather(
                                out_ap=gb[:].rearrange(
                                    "p (n d) -> p n d", d=1),
                                in_ap=table[:, 0:nel].rearrange(
                                    "p (n d) -> p n d", d=1),
                                idxs_ap=idx[:, t * 8:t * 8 + CHUNK * 8],
                                channels=128, num_elems=nel, d=1,
                                num_idxs=CHUNK * TILE_E)
                            gb16 = hi(gb[:])
                        j = t % CHUNK
                        jj = tt % 2
                        if jj == 0:
                            P2 = psP.tile([128, 2, 512], fp32)
                        nc.tensor.matmul(
                            P2[:, jj, 0:K * C_OUT],
                            lhsT=gb16[:, j * TILE_E:(j + 1) * TILE_E, 0],
                            rhs=wsp[:], start=True, stop=True,
                            skip_group_check=True)
                        if jj == 1:
                            Tp2 = tppool.tile([128, 2, K * C_OUT], fp16)
                            act_path = tt % 6 != 1  # 2/3 via Act copy
                            if act_path:
                                nc.scalar.activation(
                                    Tp2[:], P2[:, :, 0:K * C_OUT],
                                    mybir.ActivationFunctionType.Copy)
                            for j2 in range(2):
                                tx = t - 1 + j2
                                tv = Tp2[:, j2, :].rearrange(
                                    "p (k c) -> p k c", k=K)
                                bv = b_pm[:, tx * IDS_PER_TILE:
                                          tx * IDS_PER_TILE + K] \
                                    .unsqueeze(2) \
                                    .to_broadcast([128, K, C_OUT])
                                pv = (tv if act_path else
                                      P2[:, j2, 0:K * C_OUT].rearrange(
                                          "p (k c) -> p k c", k=K))
                                nc.vector.scalar_tensor_tensor(
                                    out=tv, in0=pv, scalar=0.0, in1=bv,
                                    op0=mybir.AluOpType.add,
                                    op1=mybir.AluOpType.mult)
                            for j2 in range(2):
                                ttx = tt - 1 + j2
                                tx = t - 1 + j2
                                for k in range(K):
                                    nc.tensor.matmul(
                                        agg[:, ttx * WIN:(ttx + 1) * WIN],
                                        lhsT=Tp2[:, j2,
                                                 k * C_OUT:(k + 1) * C_OUT],
                                        rhs=a_pm[:, tx * IDS_PER_TILE:
                                                 tx * IDS_PER_TILE + WIN],
                                        start=False,
                                        stop=(ttx == TILES_PER_GROUP - 1
                                              and k == K - 1),
                                        skip_group_check=True)
                    gcols = slice(g * GROUP, (g + 1) * GROUP)
                    if layer < 2:
                        st = spool.tile([C_OUT, 2 * GROUP], fp16)
                        nc.scalar.activation(
                            st[:].rearrange("p (n two) -> p n two",
                                            two=2)[:, :, 0],
                            agg[:], mybir.ActivationFunctionType.Relu,
                            bias=bias[:])
                        stu = st[:].bitcast(u32)
                        nc.sync.dma_start(out=bounce_in[:, gcols], in_=stu)
                        nc.sync.dma_start(out=root_tab[0:C_OUT, gcols],
                                          in_=stu)
                    else:
                        st3 = spool.tile([C_OUT, GROUP], fp32)
                        nc.scalar.activation(
                            st3[:], agg[:],
                            mybir.ActivationFunctionType.Relu, bias=bias[:])
                        nc.sync.dma_start(out=outt[:, gcols], in_=st3[:])
                if layer == 0:
                    nc.gpsimd.collective_compute(
                        "AllGather", mybir.AluOpType.bypass,
                        replica_groups=[list(range(N_CORES))],
                        ins=[bounce_in[:].opt()],
                        outs=[bounce_out[:].opt()])
                    nc.sync.dma_start(
                        out=table[0:C_OUT, 1:4 * idt + 1].rearrange(
                            "p (c s) -> p c s", c=4),
                        in_=bounce_out[0:4].rearrange("c p s -> p c s"))
                    nc.sync.dma_start(
                        out=table[64:64 + C_OUT, 1:4 * idt + 1].rearrange(
                            "p (c s) -> p c s", c=4),
                        in_=bounce_out[4:8].rearrange("c p s -> p c s"))
                    nc.sync.dma_start(out=table[32:64, :], in_=skip_d[0:32, :])
                    nc.sync.dma_start(out=table[96:128, :],
                                      in_=skip_d[32:64, :])
                    nc.sync.dma_start(out=root_tab[C_OUT:2 * C_OUT, :],
                                      in_=sown_d[:, :])
                elif layer == 1:
                    nc.gpsimd.collective_compute(
                        "AllGather", mybir.AluOpType.bypass,
                        replica_groups=[list(range(N_CORES))],
                        ins=[bounce_in[:].opt()],
                        outs=[bounce_out[:].opt()])
                    for q in range(4):
                        nc.sync.dma_start(
                            out=table[32 * q:32 * q + C_OUT,
                                      1:2 * idt + 1].rearrange(
                                "p (c s) -> p c s", c=2),
                            in_=bounce_out[2 * q:2 * q + 2].rearrange(
                                "c p s -> p c s"))
    nc.compile()
    return nc


# ----------------------------------------------------------------------------
# PJRT runner with cached executable (1 launch, 1 compile)
# ----------------------------------------------------------------------------

class PjrtRunner:
    def __init__(self, nc, n_cores):
        import jax
        import numpy as _np
        from jax.sharding import Mesh, PartitionSpec
        from jax.experimental.shard_map import shard_map
        from concourse import bass2jax as b2j
        import concourse.mybir as mybir

        b2j.install_neuronx_cc_hook()
        self.nc = nc
        self.n_cores = n_cores
        partition_name = (nc.partition_id_tensor.name
                          if nc.partition_id_tensor else None)
        in_names, out_names, out_avals, zero_outs = [], [], [], []
        for alloc in nc.m.functions[0].allocations:
            if not isinstance(alloc, mybir.MemoryLocationSet):
                continue
            name = alloc.memorylocations[0].name
            if alloc.kind == "ExternalInput":
                if name != partition_name:
                    in_names.append(name)
            elif alloc.kind == "ExternalOutput":
                out_names.append(name)
                shape = tuple(alloc.tensor_shape)
                dtype = mybir.dt.np(alloc.dtype)
                out_avals.append(jax.core.ShapedArray(shape, dtype))
                zero_outs.append(_np.zeros(shape, dtype))
        self.in_names = list(in_names)
        self.out_names = out_names
        self.zero_outs = zero_outs
        self.out_avals = out_avals
        n_params = len(in_names)
        n_outs = len(out_avals)
        all_in_names = in_names + out_names + (
            [partition_name] if partition_name else [])
        donate = tuple(range(n_params, n_params + n_outs))

        def _body(*args):
            operands = list(args)
            if partition_name is not None:
                operands.append(b2j.partition_id_tensor())
            outs = b2j._bass_exec_p.bind(
                *operands,
                out_avals=tuple(out_avals),
                in_names=tuple(all_in_names),
                out_names=tuple(out_names),
                lowering_input_output_aliases=(),
                sim_require_finite=False,
                sim_require_nnan=False,
                nc=nc,
            )
            return tuple(outs)

        devices = jax.devices()[:n_cores]
        mesh = Mesh(np.asarray(devices), ("core",))
        in_specs = (PartitionSpec("core"),) * (n_params + n_outs)
        out_specs = (PartitionSpec("core"),) * len(out_names)
        self.fn = jax.jit(
            shard_map(_body, mesh=mesh, in_specs=in_specs,
                      out_specs=out_specs, check_rep=False),
            donate_argnums=donate, keep_unused=True)

    def __call__(self, in_maps):
        per_core = [[np.asarray(m[name]) for name in self.in_names]
                    for m in in_maps]
        concat_in = [np.concatenate([per_core[c][i]
                                     for c in range(self.n_cores)], axis=0)
                     for i in range(len(self.in_names))]
        concat_zeros = [np.zeros((self.n_cores * z.shape[0], *z.shape[1:]),
                                 z.dtype) for z in self.zero_outs]
        out_arrs = self.fn(*concat_in, *concat_zeros)
        return [
            {name: np.asarray(out_arrs[i]).reshape(
                self.n_cores, *self.out_avals[i].shape)[c]
             for i, name in enumerate(self.out_names)}
            for c in range(self.n_cores)
        ]


_CACHE = {}


def _get_runner(T, idt):
    key = (T, idt)
    if key not in _CACHE:
        nc = build_bass_kernel(T, idt)
        _CACHE[key] = PjrtRunner(nc, N_CORES)
    return _CACHE[key]


def build_in_maps(inputs, cores, perms, T, idt):
    x = np.asarray(inputs["x"], np.float32)
    skip = np.asarray(inputs["skip"], np.float32)
    table_x, skip_tab, x_own, skip_own = build_tables(x, skip, perms, idt)
    (wsp1, wsp2, wsp3, rta1, rta2, rta3, b1, b2) = build_weights(
        np.asarray(inputs["W1"], np.float32),
        np.asarray(inputs["root1"], np.float32),
        np.asarray(inputs["b1"], np.float32),
        np.asarray(inputs["W2"], np.float32),
        np.asarray(inputs["root2"], np.float32),
        np.asarray(inputs["b2"], np.float32))
    in_maps = []
    for c, cc in enumerate(cores):
        in_maps.append({
            "table_x": table_x, "skip_tab": skip_tab,
            "x_own": x_own[c], "skip_own": skip_own[c],
            "idx12": cc["idx12"], "idx3": cc["idx3"],
            "a_pm": cc["a_pm"], "b_pm": cc["b_pm"],
            "wsp1": wsp1, "wsp2": wsp2, "wsp3": wsp3,
            "rta1": rta1, "rta2": rta2, "rta3": rta3,
            "bias1": b1, "bias2": b2,
        })
    return in_maps


def kernel(**inputs) -> np.ndarray:
    cores, perms, T, idt = build_static(np.asarray(inputs["edge_index"]),
                                        np.asarray(inputs["pseudo"]))
    runner = _get_runner(T, idt)
    in_maps = build_in_maps(inputs, cores, perms, T, idt)
    res = runner(in_maps)
    return unpermute([r["outt"] for r in res], cores)
